# revision 11
# baseline (speedup 1.0000x reference)
"""MaxUnpooling2D scatter kernel for Trainium2 (8 NeuronCores, batch-parallel).

Problem: inputs [16,128,128,64] f32, argmax [16,128,128,64] i32 holding
per-batch flattened indices into the [256,256,64] output space, laid out as
    argmax = ((2h+dh)*Wo + (2w+dw))*C + c,   dh,dw in {0,1}
Output [16,256,256,64] f32: each input value lands in one cell of its own
2x2 output window; the other three cells are 0. Windows are disjoint, so no
duplicate indices are possible and scatter-add degenerates to a masked
placement.

Key observation: the bit fields of argmax are disjoint —
    c = bits 0-5, dw = bit 6, w = bits 7-13, dh = bit 14, h = bits 15-21
so kernel() packs code8 = dh*2+dw on the host into a uint8 sideband
(2 MiB/core shipped to the device instead of the 8 MiB argmax), and each of
the four output slots is a single fused DVE op:
    out_rows[dh][:, :, dw, :] = (code8 == dh*2+dw) * v
via scalar_tensor_tensor(is_equal, mult). Per core the kernel is purely
DMA-bound: 42 MiB of HBM traffic (8 in + 2 code8 + 32 out) vs ~70 us of DVE
work, hidden under ~126 us of DMA at the ~358 GB/s per-core HBM roofline.

Sharding: batch dim 16 -> 2 batches per core (data parallel, fully local,
no collectives), gather by concatenation.
"""

import json

import numpy as np

import concourse.bass as bass
import concourse.mybir as mybir
from concourse.tile import TileContext
from concourse.bass_utils import run_bass_kernel_spmd

# ---- problem constants (hardcoded; kernel.py must be self-contained) ----
B, H, W, C = 16, 128, 128, 64
N_CORES = 8
B_SHARD = B // N_CORES  # 2 batches per core
Ho, Wo = 2 * H, 2 * W
WC = W * C  # 8192 free elems per input row (h on partitions)
WoC = Wo * C  # 16384 free elems per output row

CHUNK_W = 32  # input columns per chunk
NCH = W // CHUNK_W  # 4 chunks per batch
CHF = CHUNK_W * C  # 2048 free elems per input chunk (8 KB/partition)
CHF2 = 2 * CHF  # 4096 free elems per output-row chunk (16 KB/partition)


# The walrus build in this toolchain lowers at most ONE sem-wait per
# instruction ("Too many sync wait commands" in setupSyncWait otherwise).
# Tile's scheduler attaches several; split the excess onto preceding NoOps
# on the same engine at BIR-serialization time (semantically identical:
# per-engine program order preserves wait-before-execute).
_MAX_WAITS = 1


def _split_waits(bir_json_bytes: bytes) -> bytes:
    m = json.loads(bir_json_bytes)
    for f in m.get("functions", []):
        for bb in f.get("blocks", []):
            new_instructions = []
            for ins in bb.get("instructions", []):
                sync = ins.get("sync_info")
                waits = (sync or {}).get("on_wait") or []
                if len(waits) > _MAX_WAITS:
                    extra = waits[:-_MAX_WAITS]
                    sync["on_wait"] = waits[-_MAX_WAITS:]
                    for ci, start in enumerate(range(0, len(extra), _MAX_WAITS)):
                        chunk = extra[start : start + _MAX_WAITS]
                        nop = {
                            "engine": ins["engine"],
                            "ins": [],
                            "name": f"{ins['name']}_ws{ci}",
                            "opcode": "NoOp",
                            "outs": [],
                            "sync_info": {"on_update": [], "on_wait": chunk},
                        }
                        if ins.get("debug") is not None:
                            nop["debug"] = ins["debug"]
                        new_instructions.append(nop)
                new_instructions.append(ins)
            bb["instructions"] = new_instructions
    return json.dumps(m).encode()


def _build():
    nc = bass.Bass()
    x = nc.dram_tensor("x", [B_SHARD, H, WC], mybir.dt.float32, kind="ExternalInput")
    cd = nc.dram_tensor("cd", [B_SHARD, H, WC], mybir.dt.uint8, kind="ExternalInput")
    out = nc.dram_tensor(
        "out", [B_SHARD, Ho, WoC], mybir.dt.float32, kind="ExternalOutput"
    )

    with TileContext(nc) as tc:
        with tc.tile_pool(name="io", bufs=4) as io_pool, tc.tile_pool(
            name="rows", bufs=2
        ) as row_pool:
            for b in range(B_SHARD):
                # out rows r = 2h + dh as [128(h), 2(dh), WoC]; partition = h
                out_v = out[b].rearrange("(h t) f -> h t f", t=2)
                for j in range(NCH):
                    xt = io_pool.tile([H, CHF], mybir.dt.float32, tag="xt")
                    ct = io_pool.tile([H, CHF], mybir.dt.uint8, tag="ct")
                    # loads on the SP HWDGE ring
                    nc.sync.dma_start(out=xt[:], in_=x[b][:, j * CHF : (j + 1) * CHF])
                    nc.sync.dma_start(out=ct[:], in_=cd[b][:, j * CHF : (j + 1) * CHF])

                    code_v = ct[:].rearrange("p (w c) -> p w c", c=C)
                    x_v = xt[:].rearrange("p (w c) -> p w c", c=C)
                    for dh in range(2):
                        # interleaved output-row chunk [128, w, 2(dw), C]
                        row = row_pool.tile(
                            [H, CHF2],
                            mybir.dt.float32,
                            tag=f"row{dh}",
                            name=f"row{dh}_{b}_{j}",
                        )
                        row_v = row[:].rearrange("p (w t c) -> p w t c", t=2, c=C)
                        for dw in range(2):
                            # fused (code8 == k) * v in one DVE op
                            nc.vector.scalar_tensor_tensor(
                                out=row_v[:, :, dw, :],
                                in0=code_v,
                                scalar=float(dh * 2 + dw),
                                in1=x_v,
                                op0=mybir.AluOpType.is_equal,
                                op1=mybir.AluOpType.mult,
                            )
                        # stores on the ACT HWDGE ring; 16 KB contiguous
                        # per partition at 128 KB stride (row 2h+dh)
                        nc.scalar.dma_start(
                            out=out_v[:, dh, j * CHF2 : (j + 1) * CHF2],
                            in_=row[:],
                        )

    # serialization-time wait-split fix (see _split_waits)
    orig = nc.to_json_bytes

    def patched(*a, **k):
        return _split_waits(orig(*a, **k))

    nc.to_json_bytes = patched
    return nc


_nc_cache = None


def _run(inputs: np.ndarray, argmax: np.ndarray, **spmd_kwargs):
    global _nc_cache
    if _nc_cache is None:
        _nc_cache = _build()
    nc = _nc_cache

    x = np.ascontiguousarray(np.asarray(inputs, dtype=np.float32).reshape(B, H, WC))
    am = np.asarray(argmax, dtype=np.int32).reshape(B, H, WC)
    # host-side marshaling: pack the two routing bits (dw=bit6, dh=bit14)
    # into a uint8 sideband -> device reads 2 MiB/core instead of 8
    code8 = (((am >> 6) & 1) | ((am >> 13) & 2)).astype(np.uint8)

    in_maps = [
        {
            "x": x[i * B_SHARD : (i + 1) * B_SHARD],
            "cd": np.ascontiguousarray(code8[i * B_SHARD : (i + 1) * B_SHARD]),
        }
        for i in range(N_CORES)
    ]
    res = run_bass_kernel_spmd(
        nc, in_maps, core_ids=list(range(N_CORES)), **spmd_kwargs
    )
    out = np.concatenate([r["out"] for r in res.results], axis=0)
    return out.reshape(B, Ho, Wo, C), res


def kernel(inputs: np.ndarray, argmax: np.ndarray) -> np.ndarray:
    out, _ = _run(inputs, argmax)
    return out
